# revision 8
# baseline (speedup 1.0000x reference)
"""Trainium2 Bass kernel for DeepGEMM-style masked grouped GEMM (MoE).

Problem (hardcoded shapes):
  E=64 experts, MAX_M=256 tokens/expert, N=1024, K=4096, 128-block dequant
  scales, per-expert valid-token counts masked_m.

Strategy:
  - Expert-parallel over 8 NeuronCores: experts [8c, 8c+8) on core c.
  - Host folds the dequant scales (input_scale per (token, k-block),
    weight_scale per (n-block, k-block)) and the masked_m row mask into the
    operands, casts to bf16, and packs both operands K-major
    ([128 k-partitions, k-tile, free]) so each expert's operands stream to
    SBUF as single large fully-contiguous DMAs.
  - Device: per expert, out[mt] (128xN) = sum over 32 k-tiles of
    aT[kt]^T @ bT[kt] accumulated in PSUM (bf16 matmul, fp32 accumulate),
    then PSUM->SBUF bf16 copy and DMA out. Masked rows are exactly zero
    because the folded mask zeroes those activation rows.
"""

import os

import numpy as np
import ml_dtypes

E, MAX_M, N, K = 64, 256, 1024, 4096
BLK = 128
C = K // BLK  # 32 k-blocks (= k-tiles)
NB = N // BLK  # 8 n-blocks
NCORES = 8
EPC = E // NCORES  # experts per core
NH = 2  # N halves of 512 (one PSUM bank each)
MT = 2  # M tiles of 128

BF16 = ml_dtypes.bfloat16

LAST_EXEC_NS = None


def _build_nc():
    import concourse.mybir as mybir
    from concourse import bacc
    from concourse.tile import TileContext

    nc = bacc.Bacc("TRN2", target_bir_lowering=False, debug=False)
    a_d = nc.dram_tensor(
        "a", [EPC, BLK, C, MAX_M], mybir.dt.bfloat16, kind="ExternalInput"
    )
    b_d = nc.dram_tensor(
        "b", [EPC, BLK, C, N], mybir.dt.bfloat16, kind="ExternalInput"
    )
    o_d = nc.dram_tensor(
        "o", [EPC, MT, BLK, N], mybir.dt.bfloat16, kind="ExternalOutput"
    )

    with TileContext(nc) as tc:
        with (
            tc.tile_pool(name="apool", bufs=2) as apool,
            tc.tile_pool(name="bpool", bufs=2) as bpool,
            tc.tile_pool(name="opool", bufs=2) as opool,
            tc.tile_pool(name="psum", bufs=4, space="PSUM") as psum_pool,
        ):
            for i in range(EPC):
                # The walrus DIRECT2D DMA lowering in this toolchain accepts
                # at most ONE sync-wait per DMA instruction. Slot-recycled
                # tiles would put 2 waits (engine WAR + DMA lane) on the
                # load DMA, so a tiny same-engine memset touches the tile
                # first: the memset (a compute op, no wait limit) absorbs
                # the waits and the DMA follows in program order.
                a_t = apool.tile([BLK, C, MAX_M], mybir.dt.bfloat16)
                nc.gpsimd.memset(a_t[0:1, 0, 0:2], 0)
                nc.gpsimd.dma_start(out=a_t[:, :, :], in_=a_d[i, :, :, :])
                b_t = bpool.tile([BLK, C, N], mybir.dt.bfloat16)
                nc.gpsimd.memset(b_t[0:1, 0, 0:2], 0)
                nc.gpsimd.dma_start(out=b_t[:, :, :], in_=b_d[i, :, :, :])

                o_t = opool.tile([BLK, MT, N], mybir.dt.bfloat16)
                for mt in range(MT):
                    ps = [
                        psum_pool.tile(
                            [BLK, N // NH],
                            mybir.dt.float32,
                            name=f"ps{nh}",
                            tag=f"ps{nh}",
                        )
                        for nh in range(NH)
                    ]
                    for c in range(C):
                        lhsT = a_t[:, c, mt * BLK : (mt + 1) * BLK]
                        for nh in range(NH):
                            rhs = b_t[:, c, nh * (N // NH) : (nh + 1) * (N // NH)]
                            nc.tensor.matmul(
                                ps[nh][:, :],
                                lhsT,
                                rhs,
                                start=(c == 0),
                                stop=(c == C - 1),
                            )
                    # PSUM->SBUF cast copies on ACT, and the store DMA issued
                    # from ACT too: the store's RAW dep on the copies is then
                    # same-engine program order (no sem wait on the DMA).
                    for nh in range(NH):
                        nc.scalar.copy(
                            o_t[:, mt, nh * (N // NH) : (nh + 1) * (N // NH)],
                            ps[nh][:, :],
                        )
                    nc.scalar.dma_start(out=o_d[i, mt, :, :], in_=o_t[:, mt, :])
    # bacc pass pipeline: moves matmul waits to ldweights and splits
    # over-limit waits into EventSemaphore chains (HW allows 1 wait/inst).
    nc.compile()
    return nc


def kernel(input, input_scale, weight, weight_scale, masked_m):
    global LAST_EXEC_NS
    from concourse import bass_utils

    inp = np.asarray(input, dtype=np.float32)
    isc = np.asarray(input_scale, dtype=np.float32)
    w = np.asarray(weight, dtype=np.float32)
    wsc = np.asarray(weight_scale, dtype=np.float32)
    mm = np.asarray(masked_m, dtype=np.int32)

    # Fold row mask into the per-token scales: masked rows of `a` become
    # exactly zero, so those output rows are exactly zero after the GEMM.
    mask = (np.arange(MAX_M, dtype=np.int32)[None, :] < mm[:, None]).astype(
        np.float32
    )
    # a[e, m, k] = inp * isc[e, m, k//128] * mask  -> bf16
    a = (inp.reshape(E, MAX_M, C, BLK) * (isc * mask[:, :, None])[..., None]).astype(
        BF16
    )
    # pack K-major: a_packed[e, p, c, m] = a[e, m, c, p]
    a_packed = np.ascontiguousarray(a.transpose(0, 3, 2, 1))

    # b[e, n, k] = w * wsc[e, n//128, k//128]  -> bf16
    b = (w.reshape(E, NB, BLK, C, BLK) * wsc[:, :, None, :, None]).astype(BF16)
    # dims [E, nb, ni, c, p] -> b_packed[e, p, c, nb, ni] -> [E, p, c, N]
    b_packed = np.ascontiguousarray(b.transpose(0, 4, 3, 1, 2)).reshape(
        E, BLK, C, N
    )

    nc = _build_nc()

    in_maps = [
        {
            "a": a_packed[core * EPC : (core + 1) * EPC],
            "b": b_packed[core * EPC : (core + 1) * EPC],
        }
        for core in range(NCORES)
    ]

    trace = os.environ.get("BASS_KERNEL_TRACE", "") == "1"
    res = bass_utils.run_bass_kernel_spmd(
        nc, in_maps, core_ids=list(range(NCORES)), trace=trace
    )
    LAST_EXEC_NS = res.exec_time_ns

    # o[i, mt, p, n] per core; m = mt*128 + p
    out = np.concatenate([r["o"] for r in res.results], axis=0)  # [E, MT, BLK, N]
    return out.reshape(E, MAX_M, N)


# revision 11
# speedup vs baseline: 1.0411x; 1.0411x over previous
"""Trainium2 Bass kernel for DeepGEMM-style masked grouped GEMM (MoE).

Problem (hardcoded shapes):
  E=64 experts, MAX_M=256 tokens/expert, N=1024, K=4096, 128-block dequant
  scales, per-expert valid-token counts masked_m.

Strategy:
  - Expert-parallel over 8 NeuronCores: experts [8c, 8c+8) on core c.
  - Host folds the dequant scales (input_scale per (token, k-block),
    weight_scale per (n-block, k-block)) and the masked_m row mask into the
    operands, casts to bf16, and packs both operands K-major
    ([128 k-partitions, k-tile, free]) so each expert's operands stream to
    SBUF as single large fully-contiguous DMAs.
  - Device: per expert, out[mt] (128xN) = sum over 32 k-tiles of
    aT[kt]^T @ bT[kt] accumulated in PSUM (bf16 matmul, fp32 accumulate),
    then PSUM->SBUF bf16 copy and DMA out. Masked rows are exactly zero
    because the folded mask zeroes those activation rows.
"""

import os

import numpy as np
import ml_dtypes

E, MAX_M, N, K = 64, 256, 1024, 4096
BLK = 128
C = K // BLK  # 32 k-blocks (= k-tiles)
NB = N // BLK  # 8 n-blocks
NCORES = 8
EPC = E // NCORES  # experts per core
NH = 2  # N halves of 512 (one PSUM bank each)
MT = 2  # M tiles of 128

BF16 = ml_dtypes.bfloat16

LAST_EXEC_NS = None


def _build_nc(m_keep):
    """m_keep: number of m-rows shipped/computed per expert (128|192|256).
    Rows >= m_keep are masked-out (zero) for every expert; the output DRAM
    buffer is pre-zeroed by the runtime so untouched rows stay exactly 0.
    """
    import concourse.mybir as mybir
    from concourse import bacc
    from concourse.tile import TileContext

    # m-tiles: (partition_count per tile); mt0 always 128 rows.
    m_tiles = [128] * (m_keep // 128)
    if m_keep % 128:
        m_tiles.append(m_keep % 128)

    nc = bacc.Bacc("TRN2", target_bir_lowering=False, debug=False)
    a_d = nc.dram_tensor(
        "a", [EPC, BLK, C, m_keep], mybir.dt.bfloat16, kind="ExternalInput"
    )
    b_d = nc.dram_tensor(
        "b", [EPC, BLK, C, N], mybir.dt.bfloat16, kind="ExternalInput"
    )
    o_d = nc.dram_tensor(
        "o", [EPC, MT, BLK, N], mybir.dt.bfloat16, kind="ExternalOutput"
    )

    with TileContext(nc) as tc:
        with (
            tc.tile_pool(name="apool", bufs=2) as apool,
            tc.tile_pool(name="bpool", bufs=2) as bpool,
            tc.tile_pool(name="opool", bufs=2) as opool,
            tc.tile_pool(name="psum", bufs=4, space="PSUM") as psum_pool,
        ):
            for i in range(EPC):
                # The walrus DIRECT2D DMA lowering in this toolchain accepts
                # at most ONE sync-wait per DMA instruction. Slot-recycled
                # tiles would put 2 waits (engine WAR + DMA lane) on the
                # load DMA, so a tiny same-engine memset touches the tile
                # first: the memset (a compute op, no wait limit) absorbs
                # the waits and the DMA follows in program order.
                a_t = apool.tile([BLK, C, m_keep], mybir.dt.bfloat16)
                nc.gpsimd.memset(a_t[0:1, 0, 0:2], 0)
                b_t = bpool.tile([BLK, C, N], mybir.dt.bfloat16)
                nc.gpsimd.memset(b_t[0:1, 0, 0:2], 0)
                if i == 0:
                    # Chunk the first expert's loads so the first matmuls
                    # start after ~1.5 MiB instead of after the full 10 MiB.
                    nc.gpsimd.dma_start(
                        out=a_t[:, 0:8, :], in_=a_d[i, :, 0:8, :]
                    )
                    for cg in range(0, C, 4):
                        nc.gpsimd.dma_start(
                            out=b_t[:, cg : cg + 4, :],
                            in_=b_d[i, :, cg : cg + 4, :],
                        )
                        if cg == 0:
                            nc.gpsimd.dma_start(
                                out=a_t[:, 8:C, :], in_=a_d[i, :, 8:C, :]
                            )
                else:
                    nc.gpsimd.dma_start(out=a_t[:, :, :], in_=a_d[i, :, :, :])
                    nc.gpsimd.dma_start(out=b_t[:, :, :], in_=b_d[i, :, :, :])

                o_t = opool.tile([BLK, MT, N], mybir.dt.bfloat16)
                m_off = 0
                for mt, mrows in enumerate(m_tiles):
                    ps = [
                        psum_pool.tile(
                            [BLK, N // NH],
                            mybir.dt.float32,
                            name=f"ps{nh}",
                            tag=f"ps{nh}",
                        )
                        for nh in range(NH)
                    ]
                    for c in range(C):
                        lhsT = a_t[:, c, m_off : m_off + mrows]
                        for nh in range(NH):
                            rhs = b_t[:, c, nh * (N // NH) : (nh + 1) * (N // NH)]
                            nc.tensor.matmul(
                                ps[nh][:mrows, :],
                                lhsT,
                                rhs,
                                start=(c == 0),
                                stop=(c == C - 1),
                            )
                    # PSUM->SBUF cast copies on ACT, and the store DMA issued
                    # from ACT too: the store's RAW dep on the copies is then
                    # same-engine program order (no sem wait on the DMA).
                    for nh in range(NH):
                        nc.scalar.copy(
                            o_t[:mrows, mt, nh * (N // NH) : (nh + 1) * (N // NH)],
                            ps[nh][:mrows, :],
                        )
                    nc.scalar.dma_start(
                        out=o_d[i, mt, 0:mrows, :], in_=o_t[0:mrows, mt, :]
                    )
                    m_off += mrows
    # bacc pass pipeline: moves matmul waits to ldweights and splits
    # over-limit waits into EventSemaphore chains (HW allows 1 wait/inst).
    nc.compile()
    return nc


def kernel(input, input_scale, weight, weight_scale, masked_m):
    global LAST_EXEC_NS
    from concourse import bass_utils

    inp = np.asarray(input, dtype=np.float32)
    isc = np.asarray(input_scale, dtype=np.float32)
    w = np.asarray(weight, dtype=np.float32)
    wsc = np.asarray(weight_scale, dtype=np.float32)
    mm = np.asarray(masked_m, dtype=np.int32)

    # Rows >= max(masked_m) are masked-out everywhere: don't ship or compute
    # them (their outputs stay zero via the pre-zeroed output buffer).
    mmax = int(mm.max()) if mm.size else 0
    if mmax <= 128:
        m_keep = 128
    elif mmax <= 192:
        m_keep = 192
    else:
        m_keep = MAX_M

    # Fold row mask into the per-token scales: masked rows of `a` become
    # exactly zero, so those output rows are exactly zero after the GEMM.
    mask = (np.arange(m_keep, dtype=np.int32)[None, :] < mm[:, None]).astype(
        np.float32
    )
    # a[e, m, k] = inp * isc[e, m, k//128] * mask  -> bf16
    a = (
        inp[:, :m_keep].reshape(E, m_keep, C, BLK)
        * (isc[:, :m_keep] * mask[:, :, None])[..., None]
    ).astype(BF16)
    # pack K-major: a_packed[e, p, c, m] = a[e, m, c, p]
    a_packed = np.ascontiguousarray(a.transpose(0, 3, 2, 1))

    # b[e, n, k] = w * wsc[e, n//128, k//128]  -> bf16
    b = (w.reshape(E, NB, BLK, C, BLK) * wsc[:, :, None, :, None]).astype(BF16)
    # dims [E, nb, ni, c, p] -> b_packed[e, p, c, nb, ni] -> [E, p, c, N]
    b_packed = np.ascontiguousarray(b.transpose(0, 4, 3, 1, 2)).reshape(
        E, BLK, C, N
    )

    nc = _build_nc(m_keep)

    in_maps = [
        {
            "a": a_packed[core * EPC : (core + 1) * EPC],
            "b": b_packed[core * EPC : (core + 1) * EPC],
        }
        for core in range(NCORES)
    ]

    trace = os.environ.get("BASS_KERNEL_TRACE", "") == "1"
    res = bass_utils.run_bass_kernel_spmd(
        nc, in_maps, core_ids=list(range(NCORES)), trace=trace
    )
    LAST_EXEC_NS = res.exec_time_ns

    # o[i, mt, p, n] per core; m = mt*128 + p
    out = np.concatenate([r["o"] for r in res.results], axis=0)  # [E, MT, BLK, N]
    return out.reshape(E, MAX_M, N)
